# revision 6
# baseline (speedup 1.0000x reference)
"""Trainium2 Bass kernel for nn_LinearKAN (histogram_binning).

Math
----
reference computes, per (batch b, out o):

    out[b,o] = sum_i  PL_interp(x[b,i]; bp[o,i,:], val[o,i,:])

where bp is the SAME sorted grid for every (o,i) (tiled linspace).  A
piecewise-linear function on a uniform grid with knots u = 0..S (where
u = (x - bp0)/h) has an exact ReLU-basis expansion:

    f(u) = val_0 + sum_{s=0..S-1} C_s * relu(u - s)
    C_0  = val_1 - val_0
    C_s  = val_{s+1} - 2*val_s + val_{s-1}     (s >= 1)

so the whole layer becomes a bias plus 20 dense matmuls with contraction
over (segment s, in-feature i):

    out[b,o] = bias[o] + sum_s sum_i C_s[o,i] * relu(u[b,i] - s)
    bias[o]  = sum_i val[o,i,0]

Device kernel (per core, SPMD over 8 cores):
  - shard: batch into 4 quarters (B_loc=256) x out-features into 2 halves
    (O_loc=128).  No cross-device reduction.
  - compute u^T tiles [i,b] on ScalarE (one affine+relu activation),
    build g_s = relu(u - s) tiles on VectorE/ScalarE,
    accumulate out^T[o,b] = sum C_s^T g_s in PSUM via 40 matmuls
    (K=128 chunks of the (s,i) contraction, fp32r at full PE rate),
    add bias during the PSUM->SBUF move, DMA out.
Host only slices/transposes/prepares C (layout prep), no heavy math.
"""

import os
import numpy as np

import concourse.bass as bass
import concourse.mybir as mybir
import concourse.tile as tile
from concourse import bacc
from concourse.bass_utils import run_bass_kernel_spmd

# Problem shape (hardcoded per the task contract).
B, O, I, S = 1024, 256, 256, 20
N_CORES = 8
B_SPLIT, O_SPLIT = 4, 2
B_LOC, O_LOC = B // B_SPLIT, O // O_SPLIT  # 256, 128
KT = 2 * S          # 40 K-tiles of 128 over the (s, i) contraction
N_CCHUNK = 5        # C DMA'd in 5 chunks of [128, 1024] (512 KB each)
F32 = mybir.dt.float32

# Matmul operand dtype: float32r streams at full PE rate (1 cyc/row for
# N>=256) vs float32's 4 cyc/row.  Switchable for accuracy fallback.
MM_DT = mybir.dt.float32r if os.environ.get("KAN_MM_DT", "f32") == "f32r" else F32


def _build_nc(scale: float, ubias: float) -> bass.Bass:
    """Build the (SPMD-identical) single-core Bass graph."""
    nc = bacc.Bacc("TRN2", target_bir_lowering=False, debug=False)

    # Register const APs for the ScalarE activation biases we use
    # (activation() lowers float biases via nc.const_aps).
    def _reg_const(v: float):
        if (F32, v) in nc.const_aps.aps:
            return
        t = nc.alloc_sbuf_tensor(f"const-f32-{v}", [128, 1], F32)
        nc.gpsimd.memset(t.ap(), v)
        nc.const_aps.aps[(F32, v)] = t.ap()

    _reg_const(float(ubias))
    for kt in range(2 * S):
        if kt % 4 == 3:
            _reg_const(-float(kt // 2))
    nc.all_engine_barrier()

    xT = nc.declare_dram_parameter("xT", [I, B_LOC], F32, isOutput=False)
    C = nc.declare_dram_parameter("C", [128, KT * 128], F32, isOutput=False)
    biasp = nc.declare_dram_parameter("biasp", [128, 1], F32, isOutput=False)
    out = nc.declare_dram_parameter("out", [O_LOC, B_LOC], F32, isOutput=True)

    cchunk_w = KT * 128 // N_CCHUNK           # 1024
    kt_per_chunk = cchunk_w // 128            # 8

    with tile.TileContext(nc) as tc:
        with (
            tc.tile_pool(name="xt", bufs=2) as xpool,
            tc.tile_pool(name="u", bufs=2) as upool,
            tc.tile_pool(name="g", bufs=KT) as gpool,
            tc.tile_pool(name="c", bufs=N_CCHUNK) as cpool,
            tc.tile_pool(name="bias", bufs=1) as bpool,
            tc.tile_pool(name="o", bufs=1) as opool,
            tc.tile_pool(name="ps", bufs=1, space="PSUM") as pspool,
        ):
            # --- DMA in ---
            xt = []
            for ih in range(2):
                t = xpool.tile([128, B_LOC], F32, tag="xt")
                nc.sync.dma_start(t[:], xT[ih * 128:(ih + 1) * 128, :])
                xt.append(t)
            bias_sb = bpool.tile([128, 1], F32)
            nc.sync.dma_start(bias_sb[:], biasp[:])
            cchunks = []
            for cc in range(N_CCHUNK):
                t = cpool.tile([128, cchunk_w], F32, tag="c")
                nc.sync.dma_start(t[:], C[:, cc * cchunk_w:(cc + 1) * cchunk_w])
                cchunks.append(t)

            # --- u = relu(scale*x + ubias)  (ScalarE, one op per i-half) ---
            u = []
            for ih in range(2):
                t = upool.tile([128, B_LOC], F32, tag="u")
                nc.scalar.activation(
                    t[:], xt[ih][:], mybir.ActivationFunctionType.Relu,
                    bias=float(ubias), scale=float(scale),
                )
                u.append(t)

            # --- g_s = relu(u - s) + accumulate matmuls ---
            ps = pspool.tile([O_LOC, B_LOC], F32)
            for kt in range(KT):
                s, ih = kt // 2, kt % 2
                if s == 0:
                    rhs = u[ih]  # relu(u - 0) == u  (u >= 0)
                else:
                    rhs = gpool.tile([128, B_LOC], F32, tag="g")
                    if kt % 4 == 3:  # ~1/4 of builds on ScalarE, rest VectorE
                        nc.scalar.activation(
                            rhs[:], u[ih][:], mybir.ActivationFunctionType.Relu,
                            bias=-float(s), scale=1.0,
                        )
                    else:
                        nc.vector.tensor_scalar(
                            rhs[:], u[ih][:], float(s), 0.0,
                            mybir.AluOpType.subtract, mybir.AluOpType.max,
                        )
                lhsT = cchunks[kt // kt_per_chunk][
                    :, (kt % kt_per_chunk) * 128:(kt % kt_per_chunk + 1) * 128
                ]
                nc.tensor.matmul(
                    ps[:], lhsT.bitcast(MM_DT), rhs[:].bitcast(MM_DT),
                    start=(kt == 0), stop=(kt == KT - 1),
                )

            # --- out = ps + bias (per-partition scalar), then DMA out ---
            out_sb = opool.tile([O_LOC, B_LOC], F32)
            nc.vector.tensor_scalar(
                out_sb[:], ps[:], bias_sb[:, 0:1], None, mybir.AluOpType.add,
            )
            nc.sync.dma_start(out[:], out_sb[:])
    nc.compile()
    return nc


_NC_CACHE: dict = {}


def _get_nc(scale: float, ubias: float) -> bass.Bass:
    key = (float(scale), float(ubias), str(MM_DT))
    if key not in _NC_CACHE:
        _NC_CACHE[key] = _build_nc(scale, ubias)
    return _NC_CACHE[key]


def kernel(x: np.ndarray, breakpoints: np.ndarray, values: np.ndarray,
           **_extra) -> np.ndarray:
    x = np.asarray(x, np.float32)
    breakpoints = np.asarray(breakpoints, np.float32)
    values = np.asarray(values, np.float32)

    # Grid affine params from the (shared) breakpoint row.
    bpr = breakpoints[0, 0].astype(np.float64)
    h = (bpr[-1] - bpr[0]) / S
    scale = float(1.0 / h)
    ubias = float(-bpr[0] / h)

    # ReLU-basis coefficients (host = layout prep + finite differences).
    Vf = values  # [O, I, S+1]
    Cs = np.empty((S, O, I), np.float32)
    Cs[0] = Vf[:, :, 1] - Vf[:, :, 0]
    for s in range(1, S):
        Cs[s] = Vf[:, :, s + 1] - 2.0 * Vf[:, :, s] + Vf[:, :, s - 1]
    bias_o = Vf[:, :, 0].sum(axis=1, dtype=np.float64).astype(np.float32)  # [O]

    # Per-core C layout [j, kt, o]: kt = 2*s + ih, j = i within half,
    # o = out-feature within this core's half.
    # Cr[s, oh, o, ih, j] view of Cs[s, O, I]:
    Cr = Cs.reshape(S, O_SPLIT, O_LOC, 2, 128)
    xT_full = np.ascontiguousarray(x.T)  # [I, B]

    in_maps = []
    for c in range(N_CORES):
        bq, oh = c % B_SPLIT, c // B_SPLIT
        xT_c = np.ascontiguousarray(xT_full[:, bq * B_LOC:(bq + 1) * B_LOC])
        # [s, o, ih, j] -> [j, s, ih, o] -> [128, KT*128]
        C_c = np.ascontiguousarray(
            Cr[:, oh].transpose(3, 0, 2, 1)
        ).reshape(128, KT * 128)
        bias_c = np.ascontiguousarray(
            bias_o[oh * O_LOC:(oh + 1) * O_LOC].reshape(128, 1)
        )
        in_maps.append({"xT": xT_c, "C": C_c, "biasp": bias_c})

    nc = _get_nc(scale, ubias)
    res = run_bass_kernel_spmd(nc, in_maps, list(range(N_CORES)))

    outf = np.empty((B, O), np.float32)
    for c in range(N_CORES):
        bq, oh = c % B_SPLIT, c // B_SPLIT
        outf[bq * B_LOC:(bq + 1) * B_LOC, oh * O_LOC:(oh + 1) * O_LOC] = \
            res.results[c]["out"].T
    return outf


if __name__ == "__main__":
    rng = np.random.default_rng(0)
    x = rng.uniform(-1, 1, (B, I)).astype(np.float32)
    bp = np.tile(np.linspace(-1, 1, S + 1, dtype=np.float32), (O, I, 1))
    v = (rng.standard_normal((O, I, S + 1)) * 0.1).astype(np.float32)
    out = kernel(x, bp, v)
    print("kernel ran, out:", out.shape, out.dtype, float(out.std()))
